# revision 5
# baseline (speedup 1.0000x reference)
"""Causal multi-head attention (B=2, S=2048, H=32, D=128) on 8 TRN2 NeuronCores.

Strategy (tensor-parallel over (batch, head) pairs — 64 pairs, 8 per core):

Host side packs per-head inputs into device-friendly layouts:
  qT, kT : [hpc, D, S]  bf16 — Q^T / K^T per head (d on partitions)
  vA     : [hpc, 128, NT*129] bf16 — V tiled [kv-tile, 129] with a ones
           column appended (col 128) so the softmax denominator falls out of
           the PV matmul as an extra output column.
  tri    : [128, 128] bf16 — tri[p, f] = 1 iff p <= f (causal keep-mask for
           diagonal 128x128 blocks in S^T layout).

Device per head:
  S^T[kv, q] tiles = K_tile^T-weights @ Q^T (PE, bf16, fp32 PSUM), packed per
  q-block (512 q columns) into PSUM banks with causal trimming; one big exp
  per PSUM wave on ACT (scale=1/sqrt(D) folded in, no max subtraction —
  scores are O(5) so exp is safe in fp32); causal diagonal fixed by a bf16
  tri-mask multiply on DVE; PV with P^T chunks as the stationary operand so
  the output lands in natural [q, d] layout and the ones column of vA
  accumulates the row sums; normalize with reciprocal + tensor_scalar on DVE.

Upper-triangle blocks are skipped entirely: exp(-1e9) underflows to exactly
0.0 in fp32, so dropping them is bit-equivalent to the reference softmax.
"""

import math

import numpy as np
import ml_dtypes

import concourse.bass as bass
import concourse.mybir as mybir
import concourse.tile as tile
from concourse import bacc

B, S, H, D = 2, 2048, 32, 128
N_CORES = 8
HPC = (B * H) // N_CORES  # head-pairs per core
VW = D + 1                # V width including the ones column
SCALE = 1.0 / math.sqrt(D)
CHUNK_OFF = (0, 129, 258, 512)  # PV output chunk offsets (chunk 3 in bank 1)
BF16 = mybir.dt.bfloat16
F32 = mybir.dt.float32


def _qblock_layout(qb):
    """Bank-packed S^T layout for q-block qb (512 q cols, kv tiles 0..4qb+3).

    Returns (tiles, nbanks, valid_cols) where tiles is a list of
    (j, col, width, c0): kv-tile j lands at packed column `col`, covering
    local q columns [c0*128, c0*128 + width). Widths are causal-trimmed for
    the 4 diagonal tiles and bin-packed so no matmul crosses a PSUM bank.
    The only slack (256 cols) trails at the very end.
    """
    tiles = []
    bank = 0
    for j in range(qb * 4):
        tiles.append((j, bank * 512, 512, 0))
        bank += 1
    d0 = qb * 4
    tiles.append((d0 + 0, bank * 512, 512, 0))
    bank += 1
    tiles.append((d0 + 1, bank * 512, 384, 1))
    tiles.append((d0 + 3, bank * 512 + 384, 128, 3))
    bank += 1
    tiles.append((d0 + 2, bank * 512, 256, 2))
    bank += 1
    return tiles, bank, (bank - 1) * 512 + 256


def build_module(hpc=HPC, s=S, wave_banks=3):
    nt = s // 128
    qnb = s // 512
    ptw = ((qnb - 1) * 4 + 3) * 512  # widest packed q-block

    nc = bacc.Bacc(trn_type="TRN2")
    qT = nc.dram_tensor("qT", [hpc, D, s], BF16, kind="ExternalInput")
    kT = nc.dram_tensor("kT", [hpc, D, s], BF16, kind="ExternalInput")
    vA = nc.dram_tensor("vA", [hpc, 128, nt * VW], BF16, kind="ExternalInput")
    tri = nc.dram_tensor("tri", [128, 128], BF16, kind="ExternalInput")
    out = nc.dram_tensor("out", [hpc, 128, nt * D], F32, kind="ExternalOutput")

    exp_fn = mybir.ActivationFunctionType.Exp

    with tile.TileContext(nc) as tc:
        with (
            tc.tile_pool(name="const", bufs=1) as cpool,
            tc.tile_pool(name="io", bufs=2) as iopool,
            tc.tile_pool(name="pt", bufs=2) as ptpool,
            tc.tile_pool(name="ps", bufs=2, space="PSUM") as pspool,
            tc.tile_pool(name="po", bufs=1, space="PSUM") as popool,
            tc.tile_pool(name="nrm", bufs=4) as npool,
        ):
            tri_sb = cpool.tile([128, 128], BF16, tag="tri", name="tri_sb")
            nc.sync.dma_start(out=tri_sb, in_=tri[:, :])
            zw = cpool.tile([128, 128], BF16, tag="zw", name="zw")
            nc.vector.memset(zw, 0.0)

            for h in range(hpc):
                qT_sb = iopool.tile([128, s], BF16, tag="qT", name=f"qT{h}")
                nc.sync.dma_start(out=qT_sb, in_=qT[h])
                kT_sb = iopool.tile([128, s], BF16, tag="kT", name=f"kT{h}")
                nc.sync.dma_start(out=kT_sb, in_=kT[h])
                vA_sb = iopool.tile([128, nt * VW], BF16, tag="vA", name=f"vA{h}")
                nc.sync.dma_start(out=vA_sb, in_=vA[h])
                out_sb = iopool.tile([128, nt * D], F32, tag="osb", name=f"osb{h}")

                for qb in range(qnb):
                    tiles, nbanks, valid = _qblock_layout(qb)
                    pt_sb = ptpool.tile([128, ptw], BF16, tag="pt", name=f"pt{h}_{qb}")

                    # PV accumulator: pre-zero both banks with zero-weight
                    # matmuls so every real PV matmul is a pure accumulate
                    # (order-independent under Tile's scheduler).
                    po = popool.tile([128, 1024], F32, tag="po", name=f"po{h}_{qb}")
                    nc.tensor.matmul(po[:, 0:512], zw, kT_sb[:, 0:512],
                                     start=True, stop=True, skip_group_check=True)
                    nc.tensor.matmul(po[:, 512:1024], zw, kT_sb[:, 0:512],
                                     start=True, stop=True, skip_group_check=True)

                    # Waves of <= wave_banks PSUM banks; within each wave:
                    # scores matmuls -> exp -> diag mask -> PV matmuls, so PE
                    # always has PV work to overlap with ACT's exp.
                    wb = 0
                    while wb < nbanks:
                        wn = min(wave_banks, nbanks - wb)
                        ps = pspool.tile(
                            [128, wave_banks * 512], F32, tag="ps",
                            name=f"ps{h}_{qb}_{wb}",
                        )
                        wave_tiles = [
                            t for t in tiles if wb * 512 <= t[1] < (wb + wn) * 512
                        ]
                        for (j, col, w, c0) in wave_tiles:
                            lcol = col - wb * 512
                            q0 = qb * 512 + c0 * 128
                            nc.tensor.matmul(
                                ps[:, lcol:lcol + w],
                                kT_sb[:, j * 128:(j + 1) * 128],
                                qT_sb[:, q0:q0 + w],
                                start=True, stop=True,
                            )
                        ext = min(wn * 512, valid - wb * 512)
                        nc.scalar.activation(
                            pt_sb[:, wb * 512: wb * 512 + ext],
                            ps[:, 0:ext],
                            exp_fn, scale=SCALE,
                        )
                        for (j, col, w, c0) in wave_tiles:
                            if j >= qb * 4:  # diagonal tile: causal mask
                                nc.vector.tensor_mul(
                                    pt_sb[:, col:col + 128],
                                    pt_sb[:, col:col + 128],
                                    tri_sb,
                                )
                        for (j, col, w, c0) in wave_tiles:
                            for c in range(c0, 4):
                                lhsT = pt_sb[:, col + (c - c0) * 128: col + (c - c0) * 128 + 128]
                                nc.tensor.matmul(
                                    po[:, CHUNK_OFF[c]:CHUNK_OFF[c] + VW],
                                    lhsT,
                                    vA_sb[:, j * VW:(j + 1) * VW],
                                    start=False, stop=False, skip_group_check=True,
                                )
                        wb += wn

                    # --- normalize: out[q, :] / sums[q]
                    for c in range(4):
                        qi = qb * 4 + c
                        rc = npool.tile([128, 1], F32, tag="rc", name=f"rc{h}_{qi}")
                        nc.vector.reciprocal(
                            rc, po[:, CHUNK_OFF[c] + D: CHUNK_OFF[c] + D + 1]
                        )
                        nc.vector.tensor_scalar_mul(
                            out_sb[:, qi * D:(qi + 1) * D],
                            po[:, CHUNK_OFF[c]:CHUNK_OFF[c] + D],
                            rc,
                        )

                nc.sync.dma_start(out=out[h], in_=out_sb)
    nc.compile()
    return nc


def _pack_inputs(xq, xk, xv, s=S, b=B, h=H):
    """Full [B,S,H,D] fp32 inputs -> per-pair device layouts (bf16)."""
    bf16 = ml_dtypes.bfloat16
    nt = s // 128
    nh = b * h
    # [B,S,H,D] -> [B,H,S,D] -> [nh, S, D]
    q = np.transpose(np.asarray(xq), (0, 2, 1, 3)).reshape(nh, s, D)
    k = np.transpose(np.asarray(xk), (0, 2, 1, 3)).reshape(nh, s, D)
    v = np.transpose(np.asarray(xv), (0, 2, 1, 3)).reshape(nh, s, D)
    qT = np.ascontiguousarray(q.transpose(0, 2, 1)).astype(bf16)  # [nh, D, S]
    kT = np.ascontiguousarray(k.transpose(0, 2, 1)).astype(bf16)
    v4 = v.reshape(nh, nt, 128, D)
    ones = np.ones((nh, nt, 128, 1), np.float32)
    vA = np.concatenate([v4, ones], axis=3)          # [nh, nt, 128, VW]
    vA = np.ascontiguousarray(vA.transpose(0, 2, 1, 3)).reshape(nh, 128, nt * VW)
    vA = vA.astype(bf16)
    tri = np.triu(np.ones((128, 128), np.float32)).astype(bf16)
    return qT, kT, vA, tri


def _unpack_output(outs, s=S, b=B, h=H):
    """Per-core [hpc, 128, NT*D] fp32 -> [B, S, H*D]."""
    nt = s // 128
    o = np.concatenate([np.asarray(x) for x in outs], axis=0)  # [nh, 128, nt*D]
    o = o.reshape(b * h, 128, nt, D).transpose(0, 2, 1, 3)     # [nh, nt, 128, D]
    o = o.reshape(b, h, s, D).transpose(0, 2, 1, 3)            # [B, S, H, D]
    return np.ascontiguousarray(o.reshape(b, s, h * D)).astype(np.float32)


_CACHE = {}


def _get_module():
    if "nc" not in _CACHE:
        _CACHE["nc"] = build_module()
    return _CACHE["nc"]


def make_in_maps(xq, xk, xv):
    qT, kT, vA, tri = _pack_inputs(xq, xk, xv)
    in_maps = []
    for core in range(N_CORES):
        sl = slice(core * HPC, (core + 1) * HPC)
        in_maps.append({
            "qT": np.ascontiguousarray(qT[sl]),
            "kT": np.ascontiguousarray(kT[sl]),
            "vA": np.ascontiguousarray(vA[sl]),
            "tri": tri,
        })
    return in_maps


def kernel(xq, xk, xv, cache_k, cache_v, mask, start_pos):
    assert int(start_pos) == 0, "kernel specialized for start_pos == 0"
    from concourse.bass_utils import run_bass_kernel_spmd

    nc = _get_module()
    in_maps = make_in_maps(xq, xk, xv)
    res = run_bass_kernel_spmd(nc, in_maps, core_ids=list(range(N_CORES)))
    outs = [res.results[i]["out"] for i in range(N_CORES)]
    return _unpack_output(outs)


# revision 6
# speedup vs baseline: 1.1546x; 1.1546x over previous
"""Causal multi-head attention (B=2, S=2048, H=32, D=128) on 8 TRN2 NeuronCores.

Strategy (tensor-parallel over (batch, head) pairs — 64 pairs, 8 per core):

Host side packs per-head inputs into device-friendly layouts:
  qT, kT : [hpc, D, S]  bf16 — Q^T / K^T per head (d on partitions)
  vA     : [hpc, 128, NT*129] bf16 — V tiled [kv-tile, 129] with a ones
           column appended (col 128) so the softmax denominator falls out of
           the PV matmul as an extra output column.
  tri    : [128, 128] bf16 — tri[p, f] = 1 iff p <= f (causal keep-mask for
           diagonal 128x128 blocks in S^T layout).

Device per head:
  S^T[kv, q] tiles = K_tile^T-weights @ Q^T (PE, bf16, fp32 PSUM), packed per
  q-block (512 q columns) into PSUM banks with causal trimming; one big exp
  per PSUM wave on ACT (scale=1/sqrt(D) folded in, no max subtraction —
  scores are O(5) so exp is safe in fp32); causal diagonal fixed by a bf16
  tri-mask multiply on DVE; PV with P^T chunks as the stationary operand so
  the output lands in natural [q, d] layout and the ones column of vA
  accumulates the row sums; normalize with reciprocal + tensor_scalar on DVE.

Upper-triangle blocks are skipped entirely: exp(-1e9) underflows to exactly
0.0 in fp32, so dropping them is bit-equivalent to the reference softmax.
"""

import math

import numpy as np
import ml_dtypes

import concourse.bass as bass
import concourse.mybir as mybir
import concourse.tile as tile
from concourse import bacc

B, S, H, D = 2, 2048, 32, 128
N_CORES = 8
HPC = (B * H) // N_CORES  # head-pairs per core
VW = D + 1                # V width including the ones column
SCALE = 1.0 / math.sqrt(D)
CHUNK_OFF = (0, 129, 258, 512)  # PV output chunk offsets (chunk 3 in bank 1)
BF16 = mybir.dt.bfloat16
F32 = mybir.dt.float32


def _qblock_layout(qb):
    """Bank-packed S^T layout for q-block qb (512 q cols, kv tiles 0..4qb+3).

    Returns (tiles, nbanks, valid_cols) where tiles is a list of
    (j, col, width, c0): kv-tile j lands at packed column `col`, covering
    local q columns [c0*128, c0*128 + width). Widths are causal-trimmed for
    the 4 diagonal tiles and bin-packed so no matmul crosses a PSUM bank.
    The only slack (256 cols) trails at the very end.
    """
    tiles = []
    bank = 0
    for j in range(qb * 4):
        tiles.append((j, bank * 512, 512, 0))
        bank += 1
    d0 = qb * 4
    tiles.append((d0 + 0, bank * 512, 512, 0))
    bank += 1
    tiles.append((d0 + 1, bank * 512, 384, 1))
    tiles.append((d0 + 3, bank * 512 + 384, 128, 3))
    bank += 1
    tiles.append((d0 + 2, bank * 512, 256, 2))
    bank += 1
    return tiles, bank, (bank - 1) * 512 + 256


def build_module(hpc=HPC, s=S, wave_banks=3):
    nt = s // 128
    qnb = s // 512
    ptw = ((qnb - 1) * 4 + 3) * 512  # widest packed q-block

    nc = bacc.Bacc(trn_type="TRN2")
    qT = nc.dram_tensor("qT", [hpc, D, s], BF16, kind="ExternalInput")
    kT = nc.dram_tensor("kT", [hpc, D, s], BF16, kind="ExternalInput")
    vA = nc.dram_tensor("vA", [hpc, 128, nt * VW], BF16, kind="ExternalInput")
    tri = nc.dram_tensor("tri", [128, 128], BF16, kind="ExternalInput")
    out = nc.dram_tensor("out", [hpc, 128, nt * D], F32, kind="ExternalOutput")

    exp_fn = mybir.ActivationFunctionType.Exp

    with tile.TileContext(nc) as tc:
        with (
            tc.tile_pool(name="const", bufs=1) as cpool,
            tc.tile_pool(name="io", bufs=2) as iopool,
            tc.tile_pool(name="pt", bufs=3) as ptpool,
            tc.tile_pool(name="ps", bufs=2, space="PSUM") as pspool,
            tc.tile_pool(name="po", bufs=1, space="PSUM") as popool,
            tc.tile_pool(name="nrm", bufs=4) as npool,
            tc.tile_pool(name="un", bufs=2) as unpool,
        ):
            tri_sb = cpool.tile([128, 128], BF16, tag="tri", name="tri_sb")
            nc.sync.dma_start(out=tri_sb, in_=tri[:, :])
            zw = cpool.tile([128, 128], BF16, tag="zw", name="zw")
            nc.vector.memset(zw, 0.0)

            for h in range(hpc):
                # kT low half first: the first (descending-qb) wave needs
                # kT[:, :1536] and qT[:, (qnb-1)*512:].
                kT_sb = iopool.tile([128, s], BF16, tag="kT", name=f"kT{h}")
                nc.sync.dma_start(out=kT_sb[:, 0:s // 2], in_=kT[h][:, 0:s // 2])
                qT_sb = iopool.tile([128, s], BF16, tag="qT", name=f"qT{h}")
                nc.sync.dma_start(out=qT_sb[:, s // 2:s], in_=qT[h][:, s // 2:s])
                nc.sync.dma_start(out=kT_sb[:, s // 2:s], in_=kT[h][:, s // 2:s])
                nc.sync.dma_start(out=qT_sb[:, 0:s // 2], in_=qT[h][:, 0:s // 2])
                vA_sb = iopool.tile([128, nt * VW], BF16, tag="vA", name=f"vA{h}")
                nc.sync.dma_start(out=vA_sb, in_=vA[h])
                out_sb = iopool.tile([128, nt * D], F32, tag="osb", name=f"osb{h}")

                for qb in range(qnb - 1, -1, -1):
                    tiles, nbanks, valid = _qblock_layout(qb)
                    pt_sb = ptpool.tile([128, ptw], BF16, tag="pt", name=f"pt{h}_{qb}")

                    # PV accumulator; pre-zero exactly the columns the PV
                    # matmuls accumulate into (bank 0: chunks 0-2, bank 1:
                    # chunk 3) with zero-weight matmuls, so every real PV
                    # matmul is a pure accumulate (order-independent).
                    po = popool.tile([128, 1024], F32, tag="po", name=f"po{h}_{qb}")
                    nc.tensor.matmul(po[:, 0:CHUNK_OFF[2] + VW], zw,
                                     kT_sb[:, 0:CHUNK_OFF[2] + VW],
                                     start=True, stop=True, skip_group_check=True)
                    nc.tensor.matmul(po[:, 512:512 + VW], zw, kT_sb[:, 0:VW],
                                     start=True, stop=True, skip_group_check=True)

                    # Software pipeline: scores(w) | exp(w) on ACT | mask on
                    # DVE | PV(w-1), so PE streams wave w+1's scores while ACT
                    # runs exp(w), and PV fills PE slack one wave behind.
                    waves = []
                    wb = 0
                    while wb < nbanks:
                        wn = min(wave_banks, nbanks - wb)
                        waves.append((wb, wn, [
                            t for t in tiles if wb * 512 <= t[1] < (wb + wn) * 512
                        ]))
                        wb += wn

                    def emit_scores(wave):
                        wb, wn, wave_tiles = wave
                        ps = pspool.tile(
                            [128, wave_banks * 512], F32, tag="ps",
                            name=f"ps{h}_{qb}_{wb}",
                        )
                        for (j, col, w, c0) in wave_tiles:
                            lcol = col - wb * 512
                            q0 = qb * 512 + c0 * 128
                            nc.tensor.matmul(
                                ps[:, lcol:lcol + w],
                                kT_sb[:, j * 128:(j + 1) * 128],
                                qT_sb[:, q0:q0 + w],
                                start=True, stop=True,
                            )
                        ext = min(wn * 512, valid - wb * 512)
                        nc.scalar.activation(
                            pt_sb[:, wb * 512: wb * 512 + ext],
                            ps[:, 0:ext],
                            exp_fn, scale=SCALE,
                        )
                        for (j, col, w, c0) in wave_tiles:
                            if j >= qb * 4:  # diagonal tile: causal mask
                                nc.vector.tensor_mul(
                                    pt_sb[:, col:col + 128],
                                    pt_sb[:, col:col + 128],
                                    tri_sb,
                                )

                    def emit_pv(wave):
                        _, _, wave_tiles = wave
                        for (j, col, w, c0) in wave_tiles:
                            for c in range(c0, 4):
                                lhsT = pt_sb[:, col + (c - c0) * 128: col + (c - c0) * 128 + 128]
                                nc.tensor.matmul(
                                    po[:, CHUNK_OFF[c]:CHUNK_OFF[c] + VW],
                                    lhsT,
                                    vA_sb[:, j * VW:(j + 1) * VW],
                                    start=False, stop=False, skip_group_check=True,
                                )

                    for wi, wave in enumerate(waves):
                        emit_scores(wave)
                        if wi > 0:
                            emit_pv(waves[wi - 1])
                    emit_pv(waves[-1])

                    # Drain po fast with one copy (frees both banks for the
                    # next q-block), then normalize from SBUF off the
                    # critical path.
                    un = unpool.tile([128, 1024], F32, tag="un", name=f"un{h}_{qb}")
                    nc.vector.tensor_copy(un[:, 0:512 + VW], po[:, 0:512 + VW])
                    for c in range(4):
                        qi = qb * 4 + c
                        rc = npool.tile([128, 1], F32, tag="rc", name=f"rc{h}_{qi}")
                        nc.vector.reciprocal(
                            rc, un[:, CHUNK_OFF[c] + D: CHUNK_OFF[c] + D + 1]
                        )
                        nc.vector.tensor_scalar_mul(
                            out_sb[:, qi * D:(qi + 1) * D],
                            un[:, CHUNK_OFF[c]:CHUNK_OFF[c] + D],
                            rc,
                        )

                nc.sync.dma_start(out=out[h], in_=out_sb)
    nc.compile()
    return nc


def _pack_inputs(xq, xk, xv, s=S, b=B, h=H):
    """Full [B,S,H,D] fp32 inputs -> per-pair device layouts (bf16)."""
    bf16 = ml_dtypes.bfloat16
    nt = s // 128
    nh = b * h
    # [B,S,H,D] -> [B,H,S,D] -> [nh, S, D]
    q = np.transpose(np.asarray(xq), (0, 2, 1, 3)).reshape(nh, s, D)
    k = np.transpose(np.asarray(xk), (0, 2, 1, 3)).reshape(nh, s, D)
    v = np.transpose(np.asarray(xv), (0, 2, 1, 3)).reshape(nh, s, D)
    qT = np.ascontiguousarray(q.transpose(0, 2, 1)).astype(bf16)  # [nh, D, S]
    kT = np.ascontiguousarray(k.transpose(0, 2, 1)).astype(bf16)
    v4 = v.reshape(nh, nt, 128, D)
    ones = np.ones((nh, nt, 128, 1), np.float32)
    vA = np.concatenate([v4, ones], axis=3)          # [nh, nt, 128, VW]
    vA = np.ascontiguousarray(vA.transpose(0, 2, 1, 3)).reshape(nh, 128, nt * VW)
    vA = vA.astype(bf16)
    tri = np.triu(np.ones((128, 128), np.float32)).astype(bf16)
    return qT, kT, vA, tri


def _unpack_output(outs, s=S, b=B, h=H):
    """Per-core [hpc, 128, NT*D] fp32 -> [B, S, H*D]."""
    nt = s // 128
    o = np.concatenate([np.asarray(x) for x in outs], axis=0)  # [nh, 128, nt*D]
    o = o.reshape(b * h, 128, nt, D).transpose(0, 2, 1, 3)     # [nh, nt, 128, D]
    o = o.reshape(b, h, s, D).transpose(0, 2, 1, 3)            # [B, S, H, D]
    return np.ascontiguousarray(o.reshape(b, s, h * D)).astype(np.float32)


_CACHE = {}


def _get_module():
    if "nc" not in _CACHE:
        _CACHE["nc"] = build_module()
    return _CACHE["nc"]


def make_in_maps(xq, xk, xv):
    qT, kT, vA, tri = _pack_inputs(xq, xk, xv)
    in_maps = []
    for core in range(N_CORES):
        sl = slice(core * HPC, (core + 1) * HPC)
        in_maps.append({
            "qT": np.ascontiguousarray(qT[sl]),
            "kT": np.ascontiguousarray(kT[sl]),
            "vA": np.ascontiguousarray(vA[sl]),
            "tri": tri,
        })
    return in_maps


def kernel(xq, xk, xv, cache_k, cache_v, mask, start_pos):
    assert int(start_pos) == 0, "kernel specialized for start_pos == 0"
    from concourse.bass_utils import run_bass_kernel_spmd

    nc = _get_module()
    in_maps = make_in_maps(xq, xk, xv)
    res = run_bass_kernel_spmd(nc, in_maps, core_ids=list(range(N_CORES)))
    outs = [res.results[i]["out"] for i in range(N_CORES)]
    return _unpack_output(outs)
